# revision 13
# baseline (speedup 1.0000x reference)
"""Mamba-style AdaptedMixer SSM kernel for 8 Trainium2 NeuronCores.

Sharding: intermediate_size I=1536 split 8 ways (192 channels/core).
Collectives: AllReduce of x_proj partials [80, B*S]; ReduceScatter of
out_proj partials [B*S, Dm] (each core emits rows c*512..(c+1)*512).

Per-core layout: channel-major tiles [channels, B*S], u-col = b*S+s.
SSM scan in "quad" layout: 24 quads x 8 channels; partition p = j*16 + n
(j = channel-in-quad, n = SSM state). Recurrence via the DVE hardware scan
(tensor_tensor_scan): state = dA*state + dBu along the free (time) axis.

dA = exp(A*dt) computed as Exp(dtcrep*A + A*ln2) on ACT, where dtc = dt-ln2
stays small so its bf16 rounding keeps the exponent error ~1e-3. Row
replications across the 16 n-partitions are DMA reads from DRAM with
stride-0 AP dims; the n-reduction y = sum_n C*s is a PE accumulation with
0/1 block lhsT variants into a dense [128, S] PSUM tile.
"""

import contextlib

import numpy as np
import ml_dtypes

import concourse.bass as bass
import concourse.mybir as mybir
import concourse.tile as tile
from concourse import bacc
from concourse.bass_utils import run_bass_kernel_spmd

BF16 = ml_dtypes.bfloat16
F32 = np.float32
C0 = 0.6931471805599453

NCORES = 8
B, S, DM = 2, 2048, 768
I, N, R, KW = 1536, 16, 48, 4
IL = I // NCORES            # 192 channels per core
BS = B * S                  # 4096
NQ = IL // 8                # 24 quads
HPW = 2 * (S + 3)           # conv-padded hidden width

LAST_RUN = {}
SIM_COMPAT = False   # True: substitute Silu/Softplus for CoreSim validation


def _build_nc():
    nc = bacc.Bacc(None, target_bir_lowering=False)
    f32, bf = mybir.dt.float32, mybir.dt.bfloat16
    AF = mybir.ActivationFunctionType
    OP = mybir.AluOpType

    def silu(pool, dst, src_ap, bias=None):
        """dst = Silu(src + bias). Sim-compat: sigmoid+mult."""
        if not SIM_COMPAT:
            if bias is None:
                nc.scalar.activation(dst, src_ap, AF.Silu)
            else:
                nc.scalar.activation(dst, src_ap, AF.Silu, bias=bias)
            return
        p, fs = dst.shape[0], dst.shape[-1]
        pre = pool.tile([128, fs], f32, name="silupre", tag="_silupre")
        sg_ = pool.tile([128, fs], f32, name="silusig", tag="_silusig")
        if bias is None:
            nc.scalar.copy(pre[:p, :], src_ap)
        else:
            nc.vector.tensor_scalar_add(pre[:p, :], src_ap, bias)
        nc.scalar.activation(sg_[:p, :], pre[:p, :], AF.Sigmoid)
        nc.vector.tensor_tensor(dst, pre[:p, :], sg_[:p, :], OP.mult)

    def softplus(pool, dst, src_ap, bias):
        """dst = ln(1 + exp(src + bias)); no Softplus LUT on this
        toolchain, and z is tiny so the composition is exact."""
        p, fs = dst.shape[0], dst.shape[-1]
        e_ = pool.tile([128, fs], f32, name="spexp", tag="_spexp")
        nc.scalar.activation(e_[:p, :], src_ap, AF.Exp, bias=bias)
        nc.scalar.activation(dst, e_[:p, :], AF.Ln, bias=1.0)

    # ---- DRAM parameters (host-prepped shards) ----
    xT_d = nc.declare_dram_parameter("xT", [DM, BS], bf, isOutput=False)
    win_d = nc.declare_dram_parameter("w_in", [DM, 512], bf, isOutput=False)
    wx_d = nc.declare_dram_parameter("w_x", [IL, 80], bf, isOutput=False)
    wdt_d = nc.declare_dram_parameter("w_dt", [R, IL], bf, isOutput=False)
    wo_d = nc.declare_dram_parameter("w_oT", [IL, DM], bf, isOutput=False)
    cw_d = nc.declare_dram_parameter("conv_w4", [IL, KW], f32, isOutput=False)
    cb_d = nc.declare_dram_parameter("conv_b", [IL, 1], f32, isOutput=False)
    db_d = nc.declare_dram_parameter("dt_b", [IL, 1], f32, isOutput=False)
    al_d = nc.declare_dram_parameter("alpha", [IL, 1], f32, isOutput=False)
    fg_d = nc.declare_dram_parameter("fg", [IL, 1], f32, isOutput=False)
    dv_d = nc.declare_dram_parameter("Dv", [IL, 1], f32, isOutput=False)
    asc_d = nc.declare_dram_parameter("Ascale", [128, NQ], f32, isOutput=False)
    abi_d = nc.declare_dram_parameter("Abias", [128, NQ], f32, isOutput=False)
    # yredL: cg0 variants v=0..15 at cols v*128 (each [128,128]);
    #        cg1 variants v=0..7 at cols 2048+v*64 (each [128,64])
    yl_d = nc.declare_dram_parameter("yredL", [128, 2560], bf, isOutput=False)
    out_d = nc.declare_dram_parameter("out", [BS // NCORES, DM], f32,
                                      isOutput=True)

    with tile.TileContext(nc, num_cores=NCORES) as tc, contextlib.ExitStack() as ctx:
        consts = ctx.enter_context(tc.tile_pool(name="consts", bufs=1))
        persist = ctx.enter_context(tc.tile_pool(name="persist", bufs=1))
        dram = ctx.enter_context(tc.tile_pool(name="dram", bufs=1, space="DRAM"))

        # ---------------- constants ----------------
        win_sb = [consts.tile([128, 512], bf, name=f"win{k}", tag=f"win{k}") for k in range(6)]
        for k in range(6):
            nc.sync.dma_start(win_sb[k][:], win_d[k * 128:(k + 1) * 128, :])
        wx0 = consts.tile([128, 80], bf)
        wx1 = consts.tile([64, 80], bf)
        nc.sync.dma_start(wx0[:], wx_d[0:128, :])
        nc.sync.dma_start(wx1[:], wx_d[128:IL, :])
        wdt = consts.tile([R, IL], bf)
        nc.sync.dma_start(wdt[:], wdt_d[:])
        wo0 = consts.tile([128, DM], bf)
        wo1 = consts.tile([64, DM], bf)
        nc.sync.dma_start(wo0[:], wo_d[0:128, :])
        nc.sync.dma_start(wo1[:], wo_d[128:IL, :])
        cw = [consts.tile([128, KW], f32, name="cw0", tag="cw0"),
              consts.tile([64, KW], f32, name="cw1", tag="cw1")]
        cb = [consts.tile([128, 1], f32, name="cb0", tag="cb0"),
              consts.tile([64, 1], f32, name="cb1", tag="cb1")]
        dbt = [consts.tile([128, 1], f32, name="db0", tag="db0"),
               consts.tile([64, 1], f32, name="db1", tag="db1")]
        alp = [consts.tile([128, 1], f32, name="al0", tag="al0"),
               consts.tile([64, 1], f32, name="al1", tag="al1")]
        fgt = [consts.tile([128, 1], f32, name="fg0", tag="fg0"),
               consts.tile([64, 1], f32, name="fg1", tag="fg1")]
        dvt = [consts.tile([128, 1], f32, name="dv0", tag="dv0"),
               consts.tile([64, 1], f32, name="dv1", tag="dv1")]
        for d_, t_ in ((cw_d, cw), (cb_d, cb), (db_d, dbt), (al_d, alp),
                       (fg_d, fgt), (dv_d, dvt)):
            nc.sync.dma_start(t_[0][:], d_[0:128, :])
            nc.sync.dma_start(t_[1][:], d_[128:IL, :])
        asc = consts.tile([128, NQ], f32)
        abi = consts.tile([128, NQ], f32)
        nc.sync.dma_start(asc[:], asc_d[:])
        nc.sync.dma_start(abi[:], abi_d[:])
        yredL = consts.tile([128, 2560], bf)
        nc.sync.dma_start(yredL[:], yl_d[:])

        # ---------------- persistent activations ----------------
        sg = [persist.tile([128, BS], bf, name="sg0", tag="sg0"),
              persist.tile([64, BS], bf, name="sg1", tag="sg1")]
        h_t = [persist.tile([128, BS], bf, name="h0", tag="h0"),
               persist.tile([64, BS], bf, name="h1", tag="h1")]
        y_t = [persist.tile([128, BS], f32, name="y0", tag="y0"),
               persist.tile([64, BS], f32, name="y1", tag="y1")]
        y2 = [persist.tile([128, BS], bf, name="y20", tag="y20"),
              persist.tile([64, BS], bf, name="y21", tag="y21")]
        brep = persist.tile([128, BS], bf, name="brep", tag="brep")
        crep = persist.tile([128, BS], bf, name="crep", tag="crep")

        dtc_dram = dram.tile([IL, BS], bf, name="dtc_dram", tag="dtc_dram")
        dth_dram = dram.tile([IL, BS], bf, name="dth_dram", tag="dth_dram")
        ar_in = dram.tile([80, BS], f32, name="ar_in", tag="ar_in")
        ar_out = dram.tile([80, BS], f32, name="ar_out", tag="ar_out", addr_space="Shared")
        rs_in = dram.tile([BS, DM], bf, name="rs_in", tag="rs_in")
        rs_out = dram.tile([BS // NCORES, DM], bf, name="rs_out", tag="rs_out")

        # ================ in_proj + conv ================
        with tc.tile_pool(name="hidpool", bufs=1) as hidpool:
            hidp = [hidpool.tile([128, HPW], f32, name="hidp0", tag="hidp0"),
                    hidpool.tile([64, HPW], f32, name="hidp1", tag="hidp1")]
            for t_ in hidp:
                nc.vector.memset(t_[:], 0.0)
            with tc.tile_pool(name="xt", bufs=1) as xt_pool, \
                 tc.tile_pool(name="ppool_sb", bufs=1) as ppool_sb, \
                 tc.tile_pool(name="psum_in", bufs=2, space="PSUM") as ppool:
                xt_sb = []
                for k in range(6):
                    t_ = xt_pool.tile([128, BS], bf, name=f"xt{k}", tag=f"xt{k}")
                    nc.sync.dma_start(t_[:], xT_d[k * 128:(k + 1) * 128, :])
                    xt_sb.append(t_)
                # Mtile cols of w_in (512):
                # m0: hid ch 0..127          -> hidp0
                # m1: hid 128..191 | gate 64..127 -> hidp1[:64] / sg0[64:]
                # m2: gate 0..63   | zeros   -> sg0[:64]
                # m3: gate 128..191| zeros   -> sg1[:64]
                for m in range(4):
                    for cc in range(2):
                        ps = ppool.tile([128, S], f32, name="psin", tag="psin")
                        for c5 in range(4):
                            c5s = slice(c5 * 512, (c5 + 1) * 512)
                            for k in range(6):
                                nc.tensor.matmul(
                                    ps[:, c5s], win_sb[k][:, m * 128:(m + 1) * 128],
                                    xt_sb[k][:, cc * S + c5 * 512:cc * S + (c5 + 1) * 512],
                                    start=(k == 0), stop=(k == 5))
                        po = 3 + cc * (S + 3)
                        sl = slice(cc * S, (cc + 1) * S)
                        if m == 0:
                            nc.scalar.copy(hidp[0][:, po:po + S], ps[:])
                        elif m == 1:
                            nc.scalar.copy(hidp[1][:64, po:po + S], ps[0:64, :])
                            silu(ppool_sb, sg[0][64:128, sl], ps[64:128, :])
                        elif m == 2:
                            silu(ppool_sb, sg[0][0:64, sl], ps[0:64, :])
                        else:
                            silu(ppool_sb, sg[1][0:64, sl], ps[0:64, :])

            # causal depthwise conv + bias + SiLU -> h (bf16)
            with tc.tile_pool(name="conv", bufs=2) as cvp:
                for cg in range(2):
                    pn = 128 if cg == 0 else 64
                    for b in range(2):
                        ct = cvp.tile([128, S], f32, name="convtmp", tag="convtmp")
                        base = b * (S + 3)
                        nc.vector.tensor_scalar_mul(
                            ct[:pn, :], hidp[cg][:pn, base:base + S],
                            cw[cg][:pn, 0:1])
                        for k in range(1, KW):
                            nc.vector.scalar_tensor_tensor(
                                ct[:pn, :], hidp[cg][:pn, base + k:base + k + S],
                                cw[cg][:pn, k:k + 1], ct[:pn, :],
                                OP.mult, OP.add)
                        silu(cvp, h_t[cg][:pn, b * S:(b + 1) * S],
                             ct[:pn, :], bias=cb[cg][:pn, :])

        # ================ x_proj partial + AllReduce ================
        with tc.tile_pool(name="xp_sb", bufs=1) as xsb, \
             tc.tile_pool(name="psum_x", bufs=2, space="PSUM") as xpp:
            ssm_part = xsb.tile([80, BS], f32, name="ssm_part", tag="ssm_part")
            for cc in range(2):
                ps = xpp.tile([80, S], f32, name="psx", tag="psx")
                for c5 in range(4):
                    c5s = slice(c5 * 512, (c5 + 1) * 512)
                    us = slice(cc * S + c5 * 512, cc * S + (c5 + 1) * 512)
                    nc.tensor.matmul(ps[:, c5s], wx0[:], h_t[0][:, us],
                                     start=True, stop=False)
                    nc.tensor.matmul(ps[:, c5s], wx1[:], h_t[1][:64, us],
                                     start=False, stop=True)
                nc.scalar.copy(ssm_part[:, cc * S:(cc + 1) * S], ps[:])
            nc.sync.dma_start(ar_in[:], ssm_part[:])

        nc.gpsimd.collective_compute(
            "AllReduce", OP.add, replica_groups=[list(range(NCORES))],
            ins=[ar_in[:].opt()], outs=[ar_out[:].opt()])

        # ts rows -> bf16; B_rep/C_rep via stride-0 DMA + convert to bf16
        with tc.tile_pool(name="postar", bufs=1) as par:
            ts_b = par.tile([R, BS], bf, name="ts_b", tag="ts_b")
            ts_f = par.tile([R, BS], f32, name="ts_f", tag="ts_f")
            nc.sync.dma_start(ts_f[:], ar_out[0:R, :])
            nc.vector.tensor_copy(ts_b[:], ts_f[:])
            for dst, roff in ((brep, R), (crep, R + N)):
                tf = par.tile([128, BS], f32, name="bc_f32", tag="bc_f32")
                src = bass.AP(
                    tensor=ar_out[:].tensor,
                    offset=ar_out[:].offset + roff * BS,
                    ap=[[0, 8], [BS, N], [1, BS]])
                nc.sync.dma_start(tf[:], src)
                nc.vector.tensor_copy(dst[:], tf[:])

            # ---------- dt = softplus(dt_proj(ts) + b); dtc, dth ----------
            dt_f = [par.tile([128, BS], f32, name="dtf0", tag="dtf0"),
                    par.tile([64, BS], f32, name="dtf1", tag="dtf1")]
            with tc.tile_pool(name="psum_dt", bufs=2, space="PSUM") as dtp, \
                 tc.tile_pool(name="dtp_sb", bufs=1) as dtp_sb:
                for cg in range(2):
                    pn = 128 if cg == 0 else 64
                    for cc in range(2):
                        ps = dtp.tile([128, S], f32, name="psdt", tag="psdt")
                        for c5 in range(4):
                            nc.tensor.matmul(
                                ps[:pn, c5 * 512:(c5 + 1) * 512],
                                wdt[:, cg * 128:cg * 128 + pn],
                                ts_b[:, cc * S + c5 * 512:cc * S + (c5 + 1) * 512],
                                start=True, stop=True)
                        softplus(dtp_sb, dt_f[cg][:pn, cc * S:(cc + 1) * S],
                                 ps[:pn, :], dbt[cg][:pn, :])
            for cg in range(2):
                pn = 128 if cg == 0 else 64
                for col in (S - 1, BS - 1):
                    nc.vector.tensor_scalar_mul(
                        dt_f[cg][:pn, col:col + 1],
                        dt_f[cg][:pn, col:col + 1], alp[cg][:pn, :])
            with tc.tile_pool(name="ddt", bufs=1) as dd:
                for cg in range(2):
                    pn = 128 if cg == 0 else 64
                    dtc_t = dd.tile([128, BS], bf, name="dtc", tag="dtc")
                    dth_t = dd.tile([128, BS], bf, name="dth", tag="dth")
                    nc.vector.tensor_scalar_sub(dtc_t[:pn, :],
                                                dt_f[cg][:pn, :], C0)
                    nc.vector.tensor_tensor(dth_t[:pn, :], dt_f[cg][:pn, :],
                                            h_t[cg][:pn, :], OP.mult)
                    r0 = cg * 128
                    nc.sync.dma_start(dtc_dram[r0:r0 + pn, :], dtc_t[:pn, :])
                    nc.sync.dma_start(dth_dram[r0:r0 + pn, :], dth_t[:pn, :])

        # ================ SSM scan ================
        def bcast_ap(dram_t, q, cc):
            sl = dram_t[:]
            return bass.AP(
                tensor=sl.tensor,
                offset=sl.offset + (q * 8) * BS + cc * S,
                ap=[[BS, 8], [0, N], [1, S]])

        with tc.tile_pool(name="reps", bufs=3) as sp_rep, \
             tc.tile_pool(name="da", bufs=2) as sp_da, \
             tc.tile_pool(name="scan", bufs=2) as sp_s, \
             tc.tile_pool(name="psum_y", bufs=2, space="PSUM") as ypp:
            for cc in range(2):
                for cg in range(2):
                    nql = 16 if cg == 0 else 8
                    gp = 128 if cg == 0 else 64
                    yps = ypp.tile([128, S], f32, name="yps", tag="yps")
                    for v in range(nql):
                        q = cg * 16 + v
                        dtcrep = sp_rep.tile([128, S], bf, name="dtcrep", tag="dtcrep")
                        nc.sync.dma_start(dtcrep[:], bcast_ap(dtc_dram, q, cc))
                        dthrep = sp_rep.tile([128, S], bf, name="dthrep", tag="dthrep")
                        nc.sync.dma_start(dthrep[:], bcast_ap(dth_dram, q, cc))
                        da_t = sp_da.tile([128, S], f32, name="dat", tag="dat")
                        nc.scalar.activation(
                            da_t[:], dtcrep[:], AF.Exp,
                            bias=abi[:, q:q + 1], scale=asc[:, q:q + 1])
                        dbu = sp_s.tile([128, S], bf, name="dbu", tag="dbu")
                        nc.gpsimd.tensor_tensor(
                            dbu[:], dthrep[:], brep[:, cc * S:(cc + 1) * S],
                            OP.mult)
                        s_t = sp_s.tile([128, S], bf, name="st", tag="st")
                        nc.vector.tensor_tensor_scan(
                            s_t[:], da_t[:], dbu[:], 0.0, OP.mult, OP.add)
                        sc_t = sp_s.tile([128, S], bf, name="sct", tag="sct")
                        eng = nc.vector if (q % 2 == 0) else nc.gpsimd
                        eng.tensor_tensor(
                            sc_t[:], s_t[:], crep[:, cc * S:(cc + 1) * S],
                            OP.mult)
                        lsl = (slice(v * 128, (v + 1) * 128) if cg == 0 else
                               slice(2048 + v * 64, 2048 + (v + 1) * 64))
                        for c5 in range(4):
                            c5s = slice(c5 * 512, (c5 + 1) * 512)
                            nc.tensor.matmul(
                                yps[:gp, c5s], yredL[:, lsl], sc_t[:, c5s],
                                start=(v == 0), stop=(v == nql - 1))
                    nc.vector.tensor_copy(
                        y_t[cg][:gp, cc * S:(cc + 1) * S], yps[:gp, :])

        # ================ epilogue ================
        with tc.tile_pool(name="epi", bufs=2) as ep:
            for cg in range(2):
                pn = 128 if cg == 0 else 64
                for cc in range(2):
                    sl = slice(cc * S, (cc + 1) * S)
                    tmp = ep.tile([128, S], f32, name="epit", tag="epit")
                    nc.vector.scalar_tensor_tensor(
                        tmp[:pn, :], h_t[cg][:pn, sl], dvt[cg][:pn, :],
                        y_t[cg][:pn, sl], OP.mult, OP.add)
                    nc.vector.tensor_tensor(
                        y2[cg][:pn, sl], tmp[:pn, :], sg[cg][:pn, sl], OP.mult)
                for col in (S - 1, BS - 1):
                    nc.vector.tensor_scalar_mul(
                        y2[cg][:pn, col:col + 1], y2[cg][:pn, col:col + 1],
                        fgt[cg][:pn, :])

        # ================ out_proj + ReduceScatter ================
        with tc.tile_pool(name="psum_o", bufs=4, space="PSUM") as opp, \
             tc.tile_pool(name="osb", bufs=4) as osb:
            for m in range(BS // 128):
                ps = opp.tile([128, DM], f32, name="pso", tag="pso")
                for c5s, c5e in ((0, 512), (512, DM)):
                    nc.tensor.matmul(ps[:, c5s:c5e],
                                     y2[0][:, m * 128:(m + 1) * 128],
                                     wo0[:, c5s:c5e], start=True, stop=False)
                    nc.tensor.matmul(ps[:, c5s:c5e],
                                     y2[1][:64, m * 128:(m + 1) * 128],
                                     wo1[:64, c5s:c5e], start=False, stop=True)
                ot = osb.tile([128, DM], bf, name="ot", tag="ot")
                nc.scalar.copy(ot[:], ps[:])
                nc.sync.dma_start(rs_in[m * 128:(m + 1) * 128, :], ot[:])

        nc.gpsimd.collective_compute(
            "ReduceScatter", OP.add, replica_groups=[list(range(NCORES))],
            ins=[rs_in[:].opt()], outs=[rs_out[:].opt()])

        with tc.tile_pool(name="fin", bufs=4) as fin:
            for m in range(BS // NCORES // 128):
                t_b = fin.tile([128, DM], bf, name="finb", tag="finb")
                nc.sync.dma_start(t_b[:], rs_out[m * 128:(m + 1) * 128, :])
                t_f = fin.tile([128, DM], f32, name="finf", tag="finf")
                nc.vector.tensor_copy(t_f[:], t_b[:])
                nc.sync.dma_start(out_d[m * 128:(m + 1) * 128, :], t_f[:])

    return nc


def _prep_inputs(input_states, in_proj_w, conv_w, conv_b, x_proj_w,
                 dt_proj_w, dt_proj_b, A_log, D, out_proj_w, alpha,
                 feature_gating):
    xT = np.ascontiguousarray(
        np.asarray(input_states).reshape(BS, DM).T).astype(BF16)
    # yredL is identical for every core
    yredL = np.zeros((128, 2560), np.float32)
    for v in range(16):
        for p in range(128):
            yredL[p, v * 128 + v * 8 + p // 16] = 1.0
    for v in range(8):
        for p in range(128):
            yredL[p, 2048 + v * 64 + v * 8 + p // 16] = 1.0
    yredL = yredL.astype(BF16)

    in_maps = []
    for c in range(NCORES):
        sl = slice(c * IL, (c + 1) * IL)
        W_h = in_proj_w[sl, :]
        W_g = in_proj_w[I + c * IL: I + (c + 1) * IL, :]
        w_in = np.zeros((DM, 512), np.float32)
        w_in[:, 0:128] = W_h[0:128].T
        w_in[:, 128:192] = W_h[128:192].T
        w_in[:, 192:256] = W_g[64:128].T
        w_in[:, 256:320] = W_g[0:64].T
        w_in[:, 384:448] = W_g[128:192].T

        A = -np.exp(A_log[sl].astype(np.float64))          # [IL, N]
        Ascale = np.zeros((128, NQ), np.float64)
        for p in range(128):
            j, n = p // 16, p % 16
            Ascale[p, :] = A[np.arange(NQ) * 8 + j, n]
        Abias = (Ascale * C0).astype(F32)

        in_maps.append({
            "xT": xT,
            "w_in": w_in.astype(BF16),
            "w_x": np.ascontiguousarray(x_proj_w[:, sl].T).astype(BF16),
            "w_dt": np.ascontiguousarray(dt_proj_w[sl, :].T).astype(BF16),
            "w_oT": np.ascontiguousarray(out_proj_w[:, sl].T).astype(BF16),
            "conv_w4": np.ascontiguousarray(conv_w[sl, 0, :]).astype(F32),
            "conv_b": conv_b[sl].reshape(IL, 1).astype(F32),
            "dt_b": dt_proj_b[sl].reshape(IL, 1).astype(F32),
            "alpha": alpha[sl].reshape(IL, 1).astype(F32),
            "fg": feature_gating[sl].reshape(IL, 1).astype(F32),
            "Dv": D[sl].reshape(IL, 1).astype(F32),
            "Ascale": Ascale.astype(F32),
            "Abias": Abias,
            "yredL": yredL,
        })
    return in_maps


def _install_ntff_hook():
    """antenv.axon_hooks is absent in this image; synthesize it so
    run_bass_kernel_spmd(trace=True) can profile via the boot's ctypes
    path. Best-effort: failures degrade to no tracing."""
    import sys, types
    if "antenv.axon_hooks" in sys.modules:
        return
    try:
        from trn_agent_boot.trn_boot import _ntff_profile_via_ctypes
        hook = _ntff_profile_via_ctypes("/opt/axon/libaxon_pjrt.so")
        mod = types.ModuleType("antenv.axon_hooks")
        mod._hook = hook

        def set_axon_ntff_profile_hook(h):
            mod._hook = h

        def get_axon_ntff_profile_hook():
            return mod._hook

        mod.set_axon_ntff_profile_hook = set_axon_ntff_profile_hook
        mod.get_axon_ntff_profile_hook = get_axon_ntff_profile_hook
        sys.modules["antenv.axon_hooks"] = mod
    except Exception:
        pass


def kernel(**inputs):
    inputs = {k: np.asarray(v) for k, v in inputs.items()}
    in_maps = _prep_inputs(**inputs)
    nc = _build_nc()
    nc.compile()
    _install_ntff_hook()
    try:
        res = run_bass_kernel_spmd(nc, in_maps, core_ids=list(range(NCORES)),
                                   trace=True)
    except Exception:
        res = run_bass_kernel_spmd(nc, in_maps, core_ids=list(range(NCORES)),
                                   trace=False)
    LAST_RUN["exec_time_ns"] = getattr(res, "exec_time_ns", None)
    LAST_RUN["res"] = res
    outs = [np.asarray(res.results[c]["out"]) for c in range(NCORES)]
    full = np.concatenate(outs, axis=0)
    return full.reshape(B, S, DM).astype(np.float32)


if __name__ == "__main__":
    nc = _build_nc()
    n = sum(len(b.instructions) for b in nc.main_func.blocks)
    print("build ok; instructions:", n)
